# revision 6
# baseline (speedup 1.0000x reference)
"""Causal single-head attention on 8 TRN2 NeuronCores, data-parallel over batch.

Per core (one batch element): x [T=2048, C=1024], weights [C, H=128].
  q = x@Wq + bq ; k = x@Wk + bk ; v = x@Wv + bv
  out = softmax(mask(q k^T / sqrt(H))) @ v

Layout strategy (no on-device transposes anywhere):
  - host passes x^T [C, T]; projections contract C on partitions:
      qT, kT [H, T] (stationary = W[c,h]), v [T, H] (stationary = xT[c,t128])
  - scores computed transposed, S'[s, t] = k q^T, via stationary kT[:, s128]
  - softmax sums via a ones-column appended to v: one PV matmul per t-chunk
    yields both sum_s P'[s,t] v[s,h] and sum_s P'[s,t]
  - causal: blocks above the diagonal are skipped, diagonal s-tiles compute
    only the valid t' range, one [128,128] triangular mask on the mixed chunk
  - matmul inputs bf16 (fp32 PSUM accumulation), everything else fp32
Engine split: PE matmuls; ACT exp; DVE psum->sbuf copies + epilogue;
GpSimd f32->bf16 casts + mask gen.
"""

import numpy as np

import concourse.bass as bass
import concourse.mybir as mybir
import concourse.tile as tile
from concourse.bass_utils import run_bass_kernel_spmd

F32 = mybir.dt.float32
BF16 = mybir.dt.bfloat16
AF = mybir.ActivationFunctionType

B, T, C, H = 8, 2048, 1024, 128
P = 128
CT = C // P        # 8 contraction tiles
TBLK = 512         # t-block / projection chunk width
NBLK = T // TBLK   # 4
NST = T // P       # 16 s-tiles
SCALE = 1.0 / float(np.sqrt(H))

N_CORES = 8


def _split_multiwaits(nc, max_waits=1):
    """walrus in this image rejects >1 sem wait on one instruction; hoist
    extras onto single-wait NOPs placed just before on the same engine."""
    n_new = 0
    for fn in nc.m.functions:
        for bb in fn.blocks:
            new_insts = []
            for ins in bb.instructions:
                si = ins.sync_info
                if si is not None and si.on_wait and len(si.on_wait) > max_waits:
                    waits = list(si.on_wait)
                    for w in waits[:-max_waits]:
                        n_new += 1
                        new_insts.append(
                            mybir.InstNoOp(
                                name=f"I-waitsplit-{n_new}",
                                engine=ins.engine,
                                ins=[],
                                outs=[],
                                sync_info=mybir.SyncInfo(on_wait=[w], on_update=[]),
                            )
                        )
                    ins.sync_info = mybir.SyncInfo(
                        on_wait=waits[-max_waits:],
                        on_update=list(si.on_update or []),
                    )
                new_insts.append(ins)
            bb.instructions = new_insts
    return n_new


def _build(split=True, with_bias=False):
    nc = bass.Bass()
    xT = nc.declare_dram_parameter("xT", [C, T], F32, isOutput=False)
    wqkv = nc.declare_dram_parameter("wqkv", [C, 3 * H], F32, isOutput=False)
    if with_bias:
        bqk = nc.declare_dram_parameter("bqk", [H, 2], F32, isOutput=False)
        bv = nc.declare_dram_parameter("bv", [H], F32, isOutput=False)
    out = nc.declare_dram_parameter("out", [T, H], F32, isOutput=True)

    with (
        tile.TileContext(nc) as tc,
        tc.tile_pool(name="singles", bufs=1) as singles,
        tc.tile_pool(name="wst", bufs=2) as wst,
        tc.tile_pool(name="xst", bufs=3) as xst,
        tc.tile_pool(name="xbfp", bufs=2) as xbfp,
        tc.tile_pool(name="psbp", bufs=3) as psbp,
        tc.tile_pool(name="osbp", bufs=4) as osbp,
        tc.tile_pool(name="rsbp", bufs=4) as rsbp,
        tc.tile_pool(name="ps_qk", bufs=1, space="PSUM") as ps_qk,
        tc.tile_pool(name="ps_v", bufs=1, space="PSUM") as ps_v,
        tc.tile_pool(name="ps_s", bufs=2, space="PSUM") as ps_s,
        tc.tile_pool(name="ps_o", bufs=1, space="PSUM") as ps_o,
    ):
        # ---- weights: per-c-tile DMA + cast so the first matmul starts early
        w_bf = singles.tile([P, CT, 3 * H], BF16)
        for o in range(CT):
            wf = wst.tile([P, 3 * H], F32, tag="wf")
            nc.sync.dma_start(wf[:], wqkv[o * P : (o + 1) * P, :])
            nc.gpsimd.tensor_copy(w_bf[:, o, :], wf[:])

        if with_bias:
            bqk_sb = singles.tile([P, 2], F32)
            nc.sync.dma_start(bqk_sb[:], bqk[:, :])
            bv_rep = singles.tile([P, H], F32)
            bv_ap = bv[:]
            nc.sync.dma_start(
                bv_rep[:],
                bass.AP(
                    tensor=bv_ap.tensor, offset=bv_ap.offset, ap=[[0, P], [1, H]]
                ),
            )

        # triangular mask [128,128]: mask[i, t''] = 1.0 if t'' >= i else 0.0
        mask = singles.tile([P, P], BF16)
        nc.gpsimd.memset(mask[:], 1.0)
        nc.gpsimd.affine_select(
            out=mask[:],
            in_=mask[:],
            compare_op=mybir.AluOpType.is_ge,
            fill=0.0,
            base=0,
            pattern=[[1, P]],
            channel_multiplier=-1,
        )

        qT_sb = singles.tile([P, T], BF16)   # [h, t]
        kT_sb = singles.tile([P, T], BF16)   # [h, t]
        v_sb = singles.tile([P, NST, 132], BF16)  # [s128, s-tile, h | ones]
        nc.gpsimd.memset(v_sb[:], 1.0)

        for j in range(NBLK):
            t0 = j * TBLK

            # ---- projections for t-chunk j (per-c-tile pipeline) ----
            x_bf = xbfp.tile([P, CT, TBLK], BF16, tag="x_bf")
            pq = ps_qk.tile([P, TBLK], F32, tag="pqk")
            for o in range(CT):
                xf = xst.tile([P, TBLK], F32, tag="xf")
                nc.sync.dma_start(xf[:], xT[o * P : (o + 1) * P, t0 : t0 + TBLK])
                nc.gpsimd.tensor_copy(x_bf[:, o, :], xf[:])
                nc.tensor.matmul(
                    pq[:], w_bf[:, o, 0:H], x_bf[:, o, :],
                    start=(o == 0), stop=(o == CT - 1),
                )
            if with_bias:
                nc.vector.tensor_scalar_add(
                    qT_sb[:, t0 : t0 + TBLK], pq[:], bqk_sb[:, 0:1]
                )
            else:
                nc.vector.tensor_copy(qT_sb[:, t0 : t0 + TBLK], pq[:])

            pk = ps_qk.tile([P, TBLK], F32, tag="pqk")
            for o in range(CT):
                nc.tensor.matmul(
                    pk[:], w_bf[:, o, H : 2 * H], x_bf[:, o, :],
                    start=(o == 0), stop=(o == CT - 1),
                )
            if with_bias:
                nc.vector.tensor_scalar_add(
                    kT_sb[:, t0 : t0 + TBLK], pk[:], bqk_sb[:, 1:2]
                )
            else:
                nc.vector.tensor_copy(kT_sb[:, t0 : t0 + TBLK], pk[:])

            pv = ps_v.tile([P, 4, H], F32, tag="pv")
            for m4 in range(4):
                for o in range(CT):
                    nc.tensor.matmul(
                        pv[:, m4, :],
                        x_bf[:, o, m4 * P : (m4 + 1) * P],
                        w_bf[:, o, 2 * H : 3 * H],
                        start=(o == 0), stop=(o == CT - 1),
                    )
            nc.vector.tensor_copy(v_sb[:, 4 * j : 4 * j + 4, 0:H], pv[:])

            # ---- attention for t-block j ----
            po_tiles = [
                ps_o.tile([P, 132], F32, tag=f"po{c}", name=f"po{c}")
                for c in range(4)
            ]
            n_s = 4 * (j + 1)

            def pv_mms(m, p_sb):
                r = m - 4 * j
                for c in range(max(r, 0), 4):
                    nc.tensor.matmul(
                        po_tiles[c][:, 0 : H + 1],
                        p_sb[:, c * P : (c + 1) * P],
                        v_sb[:, m, 0 : H + 1],
                        start=(m == 0), stop=(m == 4 * j + c),
                    )

            def epilogue(c):
                po = po_tiles[c]
                rec = rsbp.tile([P, 1], F32, tag="rec")
                nc.vector.reciprocal(rec[:], po[:, H : H + 1])
                o_sb = osbp.tile([P, H], F32, tag="o_sb")
                nc.vector.tensor_scalar_mul(o_sb[:], po[:, 0:H], rec[:])
                if with_bias:
                    nc.vector.tensor_add(o_sb[:], o_sb[:], bv_rep[:])
                nc.sync.dma_start(out[t0 + c * P : t0 + (c + 1) * P, :], o_sb[:])

            def post_pv(m, p_sb):
                pv_mms(m, p_sb)
                c_done = m - 4 * j
                if c_done >= 0:
                    epilogue(c_done)

            prev = None
            for m in range(n_s):
                r = m - 4 * j
                lo = P * max(r, 0)
                ps = ps_s.tile([P, TBLK], F32, tag="ps")
                nc.tensor.matmul(
                    ps[:, lo:TBLK],
                    kT_sb[:, m * P : (m + 1) * P],
                    qT_sb[:, t0 + lo : t0 + TBLK],
                    start=True, stop=True,
                )
                p_sb = psbp.tile([P, TBLK], BF16, tag="p_sb")
                nc.scalar.activation(p_sb[:, lo:TBLK], ps[:, lo:TBLK], AF.Exp, scale=SCALE)
                if r >= 0:
                    nc.vector.tensor_mul(
                        p_sb[:, lo : lo + P], p_sb[:, lo : lo + P], mask[:]
                    )
                if prev is not None:
                    post_pv(*prev)
                prev = (m, p_sb)
            post_pv(*prev)

    if split:
        _split_multiwaits(nc)
    return nc


_NC_CACHE = {}


def _get_nc(with_bias=False):
    key = bool(with_bias)
    if key not in _NC_CACHE:
        _NC_CACHE[key] = _build(with_bias=key)
    return _NC_CACHE[key]


def _prepare_in_maps(batch_x, Wq, bq, Wk, bk, Wv, bv, with_bias):
    wqkv = np.ascontiguousarray(
        np.concatenate(
            [np.asarray(Wq), np.asarray(Wk), np.asarray(Wv)], axis=1
        ).astype(np.float32)
    )
    extra = {}
    if with_bias:
        extra["bqk"] = np.ascontiguousarray(
            np.stack([np.asarray(bq), np.asarray(bk)], axis=1).astype(np.float32)
        )
        extra["bv"] = np.ascontiguousarray(np.asarray(bv).astype(np.float32))
    bx = np.asarray(batch_x)
    return [
        {
            "xT": np.ascontiguousarray(bx[i].T.astype(np.float32)),
            "wqkv": wqkv,
            **extra,
        }
        for i in range(N_CORES)
    ]


def _needs_bias(bq, bk, bv):
    return bool(np.any(np.asarray(bq)) or np.any(np.asarray(bk)) or np.any(np.asarray(bv)))


def kernel(batch_x, Wq, bq, Wk, bk, Wv, bv):
    wb = _needs_bias(bq, bk, bv)
    nc = _get_nc(with_bias=wb)
    in_maps = _prepare_in_maps(batch_x, Wq, bq, Wk, bk, Wv, bv, with_bias=wb)
    res = run_bass_kernel_spmd(nc, in_maps, core_ids=list(range(N_CORES)))
    return np.stack([res.results[i]["out"] for i in range(N_CORES)], axis=0)


# revision 7
# speedup vs baseline: 1.2446x; 1.2446x over previous
"""Causal single-head attention on 8 TRN2 NeuronCores, data-parallel over batch.

Per core (one batch element): x [T=2048, C=1024], weights [C, H=128].
  q = x@Wq + bq ; k = x@Wk + bk ; v = x@Wv + bv
  out = softmax(mask(q k^T / sqrt(H))) @ v

Layout strategy (no on-device transposes anywhere):
  - host passes x^T [C, T]; projections contract C on partitions:
      qT, kT [H, T] (stationary = W[c,h]), v [T, H] (stationary = xT[c,t128])
  - scores computed transposed, S'[s, t] = k q^T, via stationary kT[:, s128]
  - softmax sums via a ones-column appended to v: one PV matmul per t-chunk
    yields both sum_s P'[s,t] v[s,h] and sum_s P'[s,t]
  - causal: blocks above the diagonal are skipped, diagonal s-tiles compute
    only the valid t' range, one [128,128] triangular mask on the mixed chunk
  - matmul inputs bf16 (fp32 PSUM accumulation), everything else fp32
Engine split: PE matmuls; ACT exp; DVE psum->sbuf copies + epilogue;
GpSimd f32->bf16 casts + mask gen.
"""

import numpy as np

import concourse.bass as bass
import concourse.mybir as mybir
import concourse.tile as tile
from concourse.bass_utils import run_bass_kernel_spmd

F32 = mybir.dt.float32
BF16 = mybir.dt.bfloat16
AF = mybir.ActivationFunctionType

B, T, C, H = 8, 2048, 1024, 128
P = 128
CT = C // P        # 8 contraction tiles
TBLK = 512         # t-block / projection chunk width
NBLK = T // TBLK   # 4
NST = T // P       # 16 s-tiles
SCALE = 1.0 / float(np.sqrt(H))

N_CORES = 8


def _split_multiwaits(nc, max_waits=1):
    """walrus in this image rejects >1 sem wait on one instruction; hoist
    extras onto single-wait NOPs placed just before on the same engine."""
    n_new = 0
    for fn in nc.m.functions:
        for bb in fn.blocks:
            new_insts = []
            for ins in bb.instructions:
                si = ins.sync_info
                if si is not None and si.on_wait and len(si.on_wait) > max_waits:
                    waits = list(si.on_wait)
                    for w in waits[:-max_waits]:
                        n_new += 1
                        new_insts.append(
                            mybir.InstNoOp(
                                name=f"I-waitsplit-{n_new}",
                                engine=ins.engine,
                                ins=[],
                                outs=[],
                                sync_info=mybir.SyncInfo(on_wait=[w], on_update=[]),
                            )
                        )
                    ins.sync_info = mybir.SyncInfo(
                        on_wait=waits[-max_waits:],
                        on_update=list(si.on_update or []),
                    )
                new_insts.append(ins)
            bb.instructions = new_insts
    return n_new


def _build(split=True, with_bias=False):
    nc = bass.Bass()
    xT = nc.declare_dram_parameter("xT", [C, T], F32, isOutput=False)
    wqkv = nc.declare_dram_parameter("wqkv", [C, 3 * H], F32, isOutput=False)
    if with_bias:
        bqk = nc.declare_dram_parameter("bqk", [H, 2], F32, isOutput=False)
        bv = nc.declare_dram_parameter("bv", [H], F32, isOutput=False)
    out = nc.declare_dram_parameter("out", [T, H], F32, isOutput=True)

    with (
        tile.TileContext(nc) as tc,
        tc.tile_pool(name="singles", bufs=1) as singles,
        tc.tile_pool(name="wst", bufs=2) as wst,
        tc.tile_pool(name="xst", bufs=3) as xst,
        tc.tile_pool(name="xbfp", bufs=2) as xbfp,
        tc.tile_pool(name="psbp", bufs=3) as psbp,
        tc.tile_pool(name="osbp", bufs=4) as osbp,
        tc.tile_pool(name="rsbp", bufs=4) as rsbp,
        tc.tile_pool(name="ps_qk", bufs=1, space="PSUM") as ps_qk,
        tc.tile_pool(name="ps_v", bufs=1, space="PSUM") as ps_v,
        tc.tile_pool(name="ps_s", bufs=2, space="PSUM") as ps_s,
        tc.tile_pool(name="ps_o", bufs=1, space="PSUM") as ps_o,
    ):
        # ---- weights: per-c-tile DMA + cast so the first matmul starts early
        w_bf = singles.tile([P, CT, 3 * H], BF16)
        for o in range(CT):
            wf = wst.tile([P, 3 * H], F32, tag="wf")
            nc.sync.dma_start(wf[:], wqkv[o * P : (o + 1) * P, :])
            nc.gpsimd.tensor_copy(w_bf[:, o, :], wf[:])

        if with_bias:
            bqk_sb = singles.tile([P, 2], F32)
            nc.sync.dma_start(bqk_sb[:], bqk[:, :])
            bv_rep = singles.tile([P, H], F32)
            bv_ap = bv[:]
            nc.sync.dma_start(
                bv_rep[:],
                bass.AP(
                    tensor=bv_ap.tensor, offset=bv_ap.offset, ap=[[0, P], [1, H]]
                ),
            )

        # triangular mask [128,128]: mask[i, t''] = 1.0 if t'' >= i else 0.0
        mask = singles.tile([P, P], BF16)
        nc.gpsimd.memset(mask[:], 1.0)
        nc.gpsimd.affine_select(
            out=mask[:],
            in_=mask[:],
            compare_op=mybir.AluOpType.is_ge,
            fill=0.0,
            base=0,
            pattern=[[1, P]],
            channel_multiplier=-1,
        )

        qT_sb = singles.tile([P, T], BF16)   # [h, t]
        kT_sb = singles.tile([P, T], BF16)   # [h, t]
        v_sb = singles.tile([P, NST, 132], BF16)  # [s128, s-tile, h | ones]
        nc.gpsimd.memset(v_sb[:], 1.0)

        for j in range(NBLK):
            t0 = j * TBLK

            # ---- projections for t-chunk j (per-c-tile pipeline) ----
            x_bf = xbfp.tile([P, CT, TBLK], BF16, tag="x_bf")
            pq = ps_qk.tile([P, TBLK], F32, tag="pqk")
            for o in range(CT):
                xf = xst.tile([P, TBLK], F32, tag="xf")
                nc.sync.dma_start(xf[:], xT[o * P : (o + 1) * P, t0 : t0 + TBLK])
                nc.vector.tensor_copy(x_bf[:, o, :], xf[:])
                nc.tensor.matmul(
                    pq[:], w_bf[:, o, 0:H], x_bf[:, o, :],
                    start=(o == 0), stop=(o == CT - 1),
                )
            if with_bias:
                nc.scalar.activation(
                    qT_sb[:, t0 : t0 + TBLK], pq[:], AF.Identity,
                    bias=bqk_sb[:, 0:1],
                )
            else:
                nc.scalar.activation(qT_sb[:, t0 : t0 + TBLK], pq[:], AF.Copy)

            pk = ps_qk.tile([P, TBLK], F32, tag="pqk")
            for o in range(CT):
                nc.tensor.matmul(
                    pk[:], w_bf[:, o, H : 2 * H], x_bf[:, o, :],
                    start=(o == 0), stop=(o == CT - 1),
                )
            if with_bias:
                nc.scalar.activation(
                    kT_sb[:, t0 : t0 + TBLK], pk[:], AF.Identity,
                    bias=bqk_sb[:, 1:2],
                )
            else:
                nc.scalar.activation(kT_sb[:, t0 : t0 + TBLK], pk[:], AF.Copy)

            pv = ps_v.tile([P, 4, H], F32, tag="pv")
            for m4 in range(4):
                for o in range(CT):
                    nc.tensor.matmul(
                        pv[:, m4, :],
                        x_bf[:, o, m4 * P : (m4 + 1) * P],
                        w_bf[:, o, 2 * H : 3 * H],
                        start=(o == 0), stop=(o == CT - 1),
                    )
            nc.scalar.activation(v_sb[:, 4 * j : 4 * j + 4, 0:H], pv[:], AF.Copy)

            # ---- attention for t-block j ----
            po_tiles = [
                ps_o.tile([P, 132], F32, tag=f"po{c}", name=f"po{c}")
                for c in range(4)
            ]
            n_s = 4 * (j + 1)

            def pv_mms(m, p_sb):
                r = m - 4 * j
                for c in range(max(r, 0), 4):
                    nc.tensor.matmul(
                        po_tiles[c][:, 0 : H + 1],
                        p_sb[:, c * P : (c + 1) * P],
                        v_sb[:, m, 0 : H + 1],
                        start=(m == 0), stop=(m == 4 * j + c),
                    )

            def epilogue(c):
                po = po_tiles[c]
                rec = rsbp.tile([P, 1], F32, tag="rec")
                nc.vector.reciprocal(rec[:], po[:, H : H + 1])
                o_sb = osbp.tile([P, H], F32, tag="o_sb")
                nc.vector.tensor_scalar_mul(o_sb[:], po[:, 0:H], rec[:])
                if with_bias:
                    nc.vector.tensor_add(o_sb[:], o_sb[:], bv_rep[:])
                nc.sync.dma_start(out[t0 + c * P : t0 + (c + 1) * P, :], o_sb[:])

            def post_pv(m, p_sb):
                pv_mms(m, p_sb)
                c_done = m - 4 * j
                if c_done >= 0:
                    epilogue(c_done)

            prev = None
            for m in range(n_s):
                r = m - 4 * j
                lo = P * max(r, 0)
                ps = ps_s.tile([P, TBLK], F32, tag="ps")
                nc.tensor.matmul(
                    ps[:, lo:TBLK],
                    kT_sb[:, m * P : (m + 1) * P],
                    qT_sb[:, t0 + lo : t0 + TBLK],
                    start=True, stop=True,
                )
                p_sb = psbp.tile([P, TBLK], BF16, tag="p_sb")
                nc.scalar.activation(p_sb[:, lo:TBLK], ps[:, lo:TBLK], AF.Exp, scale=SCALE)
                if r >= 0:
                    nc.vector.tensor_mul(
                        p_sb[:, lo : lo + P], p_sb[:, lo : lo + P], mask[:]
                    )
                if prev is not None:
                    post_pv(*prev)
                prev = (m, p_sb)
            post_pv(*prev)

    if split:
        _split_multiwaits(nc)
    return nc


_NC_CACHE = {}


def _get_nc(with_bias=False):
    key = bool(with_bias)
    if key not in _NC_CACHE:
        _NC_CACHE[key] = _build(with_bias=key)
    return _NC_CACHE[key]


def _prepare_in_maps(batch_x, Wq, bq, Wk, bk, Wv, bv, with_bias):
    wqkv = np.ascontiguousarray(
        np.concatenate(
            [np.asarray(Wq), np.asarray(Wk), np.asarray(Wv)], axis=1
        ).astype(np.float32)
    )
    extra = {}
    if with_bias:
        extra["bqk"] = np.ascontiguousarray(
            np.stack([np.asarray(bq), np.asarray(bk)], axis=1).astype(np.float32)
        )
        extra["bv"] = np.ascontiguousarray(np.asarray(bv).astype(np.float32))
    bx = np.asarray(batch_x)
    return [
        {
            "xT": np.ascontiguousarray(bx[i].T.astype(np.float32)),
            "wqkv": wqkv,
            **extra,
        }
        for i in range(N_CORES)
    ]


def _needs_bias(bq, bk, bv):
    return bool(np.any(np.asarray(bq)) or np.any(np.asarray(bk)) or np.any(np.asarray(bv)))


def kernel(batch_x, Wq, bq, Wk, bk, Wv, bv):
    wb = _needs_bias(bq, bk, bv)
    nc = _get_nc(with_bias=wb)
    in_maps = _prepare_in_maps(batch_x, Wq, bq, Wk, bk, Wv, bv, with_bias=wb)
    res = run_bass_kernel_spmd(nc, in_maps, core_ids=list(range(N_CORES)))
    return np.stack([res.results[i]["out"] for i in range(N_CORES)], axis=0)


# revision 8
# speedup vs baseline: 1.6624x; 1.3357x over previous
"""Causal single-head attention on 8 TRN2 NeuronCores, data-parallel over batch.

Per core (one batch element): x [T=2048, C=1024], weights [C, H=128].
  q = x@Wq + bq ; k = x@Wk + bk ; v = x@Wv + bv
  out = softmax(mask(q k^T / sqrt(H))) @ v

Layout strategy (no on-device transposes anywhere):
  - host passes x^T [C, T] bf16; projections contract C on partitions:
      qT, kT [H, T] (stationary = W[c,h]), v [T, H] (stationary = xT[c,t128])
  - scores computed transposed, S'[s, t] = k q^T, via stationary kT[:, s128]
  - softmax sums via a ones-column appended to v: the PV matmul per t-chunk
    yields both sum_s P'[s,t] v[s,h] and sum_s P'[s,t]
  - causal: blocks above the diagonal are skipped, diagonal s-tiles compute
    only the valid t' range, one [128,128] triangular mask on the mixed chunk
  - projection work for chunk j+1 is emitted interleaved into attention
    block j so the PE never stalls on the softmax exp
  - matmul inputs bf16 (fp32 PSUM accumulation); output + biases fp32
"""

import numpy as np
import ml_dtypes

import concourse.bass as bass
import concourse.mybir as mybir
import concourse.tile as tile
from concourse.bass_utils import run_bass_kernel_spmd

F32 = mybir.dt.float32
BF16 = mybir.dt.bfloat16
AF = mybir.ActivationFunctionType

B, T, C, H = 8, 2048, 1024, 128
P = 128
CT = C // P        # 8 contraction tiles
TBLK = 512         # t-block / projection chunk width
NBLK = T // TBLK   # 4
NST = T // P       # 16 s-tiles
SCALE = 1.0 / float(np.sqrt(H))

N_CORES = 8


def _split_multiwaits(nc, max_waits=1):
    """walrus in this image rejects >1 sem wait on one instruction; hoist
    extras onto single-wait NOPs placed just before on the same engine."""
    n_new = 0
    for fn in nc.m.functions:
        for bb in fn.blocks:
            new_insts = []
            for ins in bb.instructions:
                si = ins.sync_info
                if si is not None and si.on_wait and len(si.on_wait) > max_waits:
                    waits = list(si.on_wait)
                    for w in waits[:-max_waits]:
                        n_new += 1
                        new_insts.append(
                            mybir.InstNoOp(
                                name=f"I-waitsplit-{n_new}",
                                engine=ins.engine,
                                ins=[],
                                outs=[],
                                sync_info=mybir.SyncInfo(on_wait=[w], on_update=[]),
                            )
                        )
                    ins.sync_info = mybir.SyncInfo(
                        on_wait=waits[-max_waits:],
                        on_update=list(si.on_update or []),
                    )
                new_insts.append(ins)
            bb.instructions = new_insts
    return n_new


def _build(split=True, with_bias=False):
    nc = bass.Bass()
    xT = nc.declare_dram_parameter("xT", [C, T], BF16, isOutput=False)
    wqkv = nc.declare_dram_parameter("wqkv", [C, 3 * H], BF16, isOutput=False)
    if with_bias:
        bqk = nc.declare_dram_parameter("bqk", [H, 2], F32, isOutput=False)
        bv = nc.declare_dram_parameter("bv", [H], F32, isOutput=False)
    out = nc.declare_dram_parameter("out", [T, H], F32, isOutput=True)

    with (
        tile.TileContext(nc) as tc,
        tc.tile_pool(name="singles", bufs=1) as singles,
        tc.tile_pool(name="xbfp", bufs=2) as xbfp,
        tc.tile_pool(name="psbp", bufs=4) as psbp,
        tc.tile_pool(name="osbp", bufs=4) as osbp,
        tc.tile_pool(name="rsbp", bufs=4) as rsbp,
        tc.tile_pool(name="ps_prj", bufs=1, space="PSUM") as ps_prj,
        tc.tile_pool(name="ps_s", bufs=3, space="PSUM") as ps_s,
        tc.tile_pool(name="ps_o", bufs=1, space="PSUM") as ps_o,
    ):
        # ---- constants ----
        w_bf = singles.tile([P, CT, 3 * H], BF16)
        nc.sync.dma_start(w_bf[:], wqkv.rearrange("(o p) n -> p o n", p=P))

        if with_bias:
            bqk_sb = singles.tile([P, 2], F32)
            nc.sync.dma_start(bqk_sb[:], bqk[:, :])
            bv_rep = singles.tile([P, H], F32)
            bv_ap = bv[:]
            nc.sync.dma_start(
                bv_rep[:],
                bass.AP(
                    tensor=bv_ap.tensor, offset=bv_ap.offset, ap=[[0, P], [1, H]]
                ),
            )

        # triangular mask [128,128]: mask[i, t''] = 1.0 if t'' >= i else 0.0
        mask = singles.tile([P, P], BF16)
        nc.gpsimd.memset(mask[:], 1.0)
        nc.gpsimd.affine_select(
            out=mask[:],
            in_=mask[:],
            compare_op=mybir.AluOpType.is_ge,
            fill=0.0,
            base=0,
            pattern=[[1, P]],
            channel_multiplier=-1,
        )

        qT_sb = singles.tile([P, T], BF16)   # [h, t]
        kT_sb = singles.tile([P, T], BF16)   # [h, t]
        v_sb = singles.tile([P, NST, 132], BF16)  # [s128, s-tile, h | ones]
        nc.gpsimd.memset(v_sb[:], 1.0)

        def gen_proj(j):
            """Projection work for t-chunk j, yielded in small PE units.

            All of q/k/v0..v3 accumulate through ONE shared PSUM bank as
            strictly sequential accumulation groups."""
            t0 = j * TBLK
            x_bf = xbfp.tile([P, CT, TBLK], BF16, tag="x_bf", name="x_bf")
            pqk = ps_prj.tile([P, TBLK], F32, tag="prj", name="pq")
            for o in range(CT):
                nc.sync.dma_start(
                    x_bf[:, o, :], xT[o * P : (o + 1) * P, t0 : t0 + TBLK]
                )
                nc.tensor.matmul(
                    pqk[:], w_bf[:, o, 0:H], x_bf[:, o, :],
                    start=(o == 0), stop=(o == CT - 1),
                )
                yield
            if with_bias:
                nc.scalar.activation(
                    qT_sb[:, t0 : t0 + TBLK], pqk[:], AF.Identity,
                    bias=bqk_sb[:, 0:1],
                )
            else:
                nc.scalar.activation(qT_sb[:, t0 : t0 + TBLK], pqk[:], AF.Copy)
            yield

            pqk = ps_prj.tile([P, TBLK], F32, tag="prj", name="pk")
            for o in range(CT):
                nc.tensor.matmul(
                    pqk[:], w_bf[:, o, H : 2 * H], x_bf[:, o, :],
                    start=(o == 0), stop=(o == CT - 1),
                )
                yield
            if with_bias:
                nc.scalar.activation(
                    kT_sb[:, t0 : t0 + TBLK], pqk[:], AF.Identity,
                    bias=bqk_sb[:, 1:2],
                )
            else:
                nc.scalar.activation(kT_sb[:, t0 : t0 + TBLK], pqk[:], AF.Copy)
            yield

            pvv = ps_prj.tile([P, 4, H], F32, tag="prj", name="pv")
            for m4 in range(4):
                for o in range(CT):
                    nc.tensor.matmul(
                        pvv[:, m4, :],
                        x_bf[:, o, m4 * P : (m4 + 1) * P],
                        w_bf[:, o, 2 * H : 3 * H],
                        start=(o == 0), stop=(o == CT - 1),
                    )
                    if o % 2 == 1:
                        yield
            nc.scalar.activation(v_sb[:, 4 * j : 4 * j + 4, 0:H], pvv[:], AF.Copy)
            yield

        # chunk 0 projections up-front
        for _ in gen_proj(0):
            pass

        for j in range(NBLK):
            t0 = j * TBLK
            nxt = gen_proj(j + 1) if j + 1 < NBLK else None
            nxt_left = 26 if nxt is not None else 0  # yields per chunk

            po_tiles = [
                ps_o.tile([P, 132], F32, tag=f"po{c}", name=f"po{c}")
                for c in range(4)
            ]
            n_s = 4 * (j + 1)

            def pv_mms(m, p_sb):
                r = m - 4 * j
                for c in range(max(r, 0), 4):
                    nc.tensor.matmul(
                        po_tiles[c][:, 0 : H + 1],
                        p_sb[:, c * P : (c + 1) * P],
                        v_sb[:, m, 0 : H + 1],
                        start=(m == 0), stop=(m == 4 * j + c),
                    )

            def epilogue(c):
                po = po_tiles[c]
                rec = rsbp.tile([P, 1], F32, tag="rec", name="rec")
                nc.vector.reciprocal(rec[:], po[:, H : H + 1])
                o_sb = osbp.tile([P, H], F32, tag="o_sb", name="o_sb")
                nc.vector.tensor_scalar_mul(o_sb[:], po[:, 0:H], rec[:])
                if with_bias:
                    nc.vector.tensor_add(o_sb[:], o_sb[:], bv_rep[:])
                nc.sync.dma_start(out[t0 + c * P : t0 + (c + 1) * P, :], o_sb[:])

            def post_pv(m, p_sb):
                pv_mms(m, p_sb)
                c_done = m - 4 * j
                if c_done >= 0:
                    epilogue(c_done)

            prev = None
            for m in range(n_s):
                r = m - 4 * j
                lo = P * max(r, 0)
                ps = ps_s.tile([P, TBLK], F32, tag="ps", name="ps")
                nc.tensor.matmul(
                    ps[:, lo:TBLK],
                    kT_sb[:, m * P : (m + 1) * P],
                    qT_sb[:, t0 + lo : t0 + TBLK],
                    start=True, stop=True,
                )
                p_sb = psbp.tile([P, TBLK], BF16, tag="p_sb", name="p_sb")
                nc.scalar.activation(
                    p_sb[:, lo:TBLK], ps[:, lo:TBLK], AF.Exp, scale=SCALE
                )
                if r >= 0:
                    nc.vector.tensor_mul(
                        p_sb[:, lo : lo + P], p_sb[:, lo : lo + P], mask[:]
                    )
                if prev is not None:
                    post_pv(*prev)
                prev = (m, p_sb)
                # interleave next chunk's projection units
                if nxt is not None:
                    k_units = -(-nxt_left // (n_s - m)) if m < n_s else nxt_left
                    for _ in range(k_units):
                        try:
                            next(nxt)
                            nxt_left -= 1
                        except StopIteration:
                            nxt = None
                            break
            post_pv(*prev)
            if nxt is not None:
                for _ in nxt:
                    pass

    if split:
        _split_multiwaits(nc)
    return nc


_NC_CACHE = {}


def _get_nc(with_bias=False):
    key = bool(with_bias)
    if key not in _NC_CACHE:
        _NC_CACHE[key] = _build(with_bias=key)
    return _NC_CACHE[key]


def _prepare_in_maps(batch_x, Wq, bq, Wk, bk, Wv, bv, with_bias):
    wqkv = np.ascontiguousarray(
        np.concatenate([np.asarray(Wq), np.asarray(Wk), np.asarray(Wv)], axis=1)
    ).astype(ml_dtypes.bfloat16)
    extra = {}
    if with_bias:
        extra["bqk"] = np.ascontiguousarray(
            np.stack([np.asarray(bq), np.asarray(bk)], axis=1).astype(np.float32)
        )
        extra["bv"] = np.ascontiguousarray(np.asarray(bv).astype(np.float32))
    bx = np.asarray(batch_x)
    return [
        {
            "xT": np.ascontiguousarray(bx[i].T).astype(ml_dtypes.bfloat16),
            "wqkv": wqkv,
            **extra,
        }
        for i in range(N_CORES)
    ]


def _needs_bias(bq, bk, bv):
    return bool(
        np.any(np.asarray(bq)) or np.any(np.asarray(bk)) or np.any(np.asarray(bv))
    )


def kernel(batch_x, Wq, bq, Wk, bk, Wv, bv):
    wb = _needs_bias(bq, bk, bv)
    nc = _get_nc(with_bias=wb)
    in_maps = _prepare_in_maps(batch_x, Wq, bq, Wk, bk, Wv, bv, with_bias=wb)
    res = run_bass_kernel_spmd(nc, in_maps, core_ids=list(range(N_CORES)))
    return np.stack([res.results[i]["out"] for i in range(N_CORES)], axis=0)
